# revision 26
# baseline (speedup 1.0000x reference)
"""Trainium2 Bass kernel for 16-head causal multi-head attention.

Problem: B=2, S=2048, D=1024, H=16 (head dim 64), causal mask.
    out = softmax((XqWq+bq)(XkWk+bk)^T / 8, causal) (XvWv+bv) Wo + bo

Sharding: tensor-parallel over heads. Each of the 8 cores owns 2 heads:
Wq/Wk/Wv column-sliced (128 cols), Wo row-sliced (128 rows). Each core
computes its heads end-to-end and produces a partial output (ctx_c @ Wo_c);
the host sums the 8 partials and adds (bv @ Wo + bo).

Device-side layout choices (per core):
  - Host passes X^T (features-major) in fp16, so projection matmuls get the
    contraction dim (features) on partitions without any device transposes.
  - Q^T, K^T are produced as [128 = 2 heads x 64 dk, 4096 tokens] fp16.
  - V is produced token-major [tok, dk] (needed as PV's stationary operand),
    augmented with a ones column so each PV matmul also yields the softmax
    denominator for free.
  - Scores are computed transposed, S^T[k, q] = K @ Q^T, so that softmax'd
    probabilities P^T already have the PV contraction dim (k) on partitions.
  - exp has no max subtraction (scores/8 are ~N(0,1); exp never overflows
    fp32), so no flash-style rescaling is needed and the softmax denominator
    can come out of the PV matmul's ones column.
  - Causal structure is exploited: fully-masked k-tiles are skipped, the
    partly-masked diagonal 128x128 sub-blocks are masked by multiplying with
    a host-provided triangular 0/1 tile.
"""

import math

import numpy as np

# Full-problem constants
B, S, D, H = 2, 2048, 1024, 16
DK = D // H  # 64
NCORES = 8
HPC = H // NCORES  # heads per core
P = 128
QC = 512  # tokens per attention q-chunk / projection chunk

_PROGRAM_CACHE = {}
TRACE = False  # set True (e.g. from test.py) to capture an NTFF profile
LAST = {}      # holds the most recent BassKernelResults


# ---------------------------------------------------------------------------
# Device program
# ---------------------------------------------------------------------------

def _mha_body(ctx, tc, io, s, d, b):
    import concourse.bass as bass
    from concourse import mybir

    F16 = mybir.dt.float16
    F32 = mybir.dt.float32
    Exp = mybir.ActivationFunctionType.Exp
    Identity = mybir.ActivationFunctionType.Identity

    nc = tc.nc
    nch = s // QC       # q chunks per sequence
    kpc = QC // P       # k tiles per chunk (4)
    nf = d // P         # feature tiles

    xq, xk, xv = io["xq_t"], io["xk_t"], io["xv_t"]
    wq, wk, wv, wo = io["wq"], io["wk"], io["wv"], io["wo"]
    bq, bk = io["bq"], io["bk"]
    tri = io["tri"]
    out_t = io["out_t"]

    consts = ctx.enter_context(tc.tile_pool(name="consts", bufs=1))
    persist = ctx.enter_context(tc.tile_pool(name="persist", bufs=1))
    xs = ctx.enter_context(tc.tile_pool(name="xs", bufs=1))
    pts = ctx.enter_context(tc.tile_pool(name="pts", bufs=3))
    ptd = ctx.enter_context(tc.tile_pool(name="ptd", bufs=1))
    rcs = ctx.enter_context(tc.tile_pool(name="rcs", bufs=2))
    wout = ctx.enter_context(tc.tile_pool(name="wout", bufs=3))
    pspool = ctx.enter_context(tc.tile_pool(name="ps", bufs=1, space="PSUM"))

    # PSUM bank map (8 banks):
    #   bk0+bk1 / bk2+bk3: double-buffered 2-bank "wide" score tiles
    #     [128, 1024] = both heads' S^T for one k-tile side by side
    #   bk4 / bk5: PV accumulators (ctx + softmax sums) per head
    #   bk6: normalize broadcast
    #   bk7: projections + output projection
    def ps_tile(tag, width=QC):
        return pspool.tile([P, width], F32, tag=tag, name=tag)

    # ---- constants -------------------------------------------------------
    wq_sb = consts.tile([P, nf, P], F16, tag="wq")
    nc.sync.dma_start(wq_sb[:], wq.rearrange("(o p) m -> p o m", p=P))
    wk_sb = consts.tile([P, nf, P], F16, tag="wk")
    nc.sync.dma_start(wk_sb[:], wk.rearrange("(o p) m -> p o m", p=P))
    wv_sb = consts.tile([P, nf, P], F16, tag="wv")
    nc.sync.dma_start(wv_sb[:], wv.rearrange("(o p) m -> p o m", p=P))
    wo_sb = consts.tile([P, d], F16, tag="wo")
    nc.sync.dma_start(wo_sb[:], wo[:, :])
    tri_sb = consts.tile([P, P], F16, tag="tri")
    nc.sync.dma_start(tri_sb[:], tri[:, :])
    bq_sb = consts.tile([P, 1], F32, tag="bq")
    nc.sync.dma_start(bq_sb[:], bq[:, :])
    bk_sb = consts.tile([P, 1], F32, tag="bk")
    nc.sync.dma_start(bk_sb[:], bk[:, :])
    ones_sb = consts.tile([P, 64], F16, tag="ones")
    nc.vector.memset(ones_sb[:], 1.0)

    qt_tiles = {}
    kt_tiles = {}
    v_tiles = {}
    diag_zeroed = set()
    pending_norm_wo = None

    for bb in range(b):
        # One big DMA per (input, feature-tile): [128, s] fp16 covering the
        # whole batch sequence (minimizes per-DMA fixed costs). bufs=1 tags:
        # the bb=1 loads naturally wait for (and overlap) bb=0's consumers.
        bx = {}
        bx0 = {}
        for nm, xsrc in (("q", xq), ("k", xk), ("v", xv)):
            for f in range(nf):
                x0 = xs.tile([P, QC], F16, tag=f"c0x{nm}{f}", name=f"c0x{nm}{f}")
                nc.sync.dma_start(x0[:], xsrc[f * P:(f + 1) * P,
                                              bb * s:bb * s + QC])
                bx0[(nm, f)] = x0
        for nm, xsrc in (("q", xq), ("k", xk), ("v", xv)):
            for f in range(nf):
                xt = xs.tile([P, s - QC], F16, tag=f"x{nm}{f}", name=f"x{nm}{f}")
                nc.sync.dma_start(xt[:], xsrc[f * P:(f + 1) * P,
                                              bb * s + QC:(bb + 1) * s])
                bx[(nm, f)] = xt

        def xsl(nm, f, lo, hi):
            """Slice tokens [lo:hi) of this batch from fast-path/wide tiles."""
            if hi <= QC:
                return bx0[(nm, f)][:, lo:hi]
            return bx[(nm, f)][:, lo - QC:hi - QC]

        for jj in range(nch):  # chunk-interleaved: project j, then attend j
            j = bb * nch + jj
            co = jj * QC

            # ---- projections for chunk j (single PSUM bank bk7) ----------
            for (nm, w_sb, b_sb, store) in (
                ("q", wq_sb, bq_sb, qt_tiles),
                ("k", wk_sb, bk_sb, kt_tiles),
            ):
                pp = ps_tile("bk7")
                for f in range(nf):
                    nc.tensor.matmul(pp[:], w_sb[:, f, :],
                                     xsl(nm, f, co, co + QC),
                                     start=(f == 0), stop=(f == nf - 1))
                t = persist.tile([P, QC], F16, tag=f"{nm}t{j}")
                nc.scalar.activation(t[:], pp[:], Identity, bias=b_sb[:, 0:1],
                                     scale=1.0)
                store[j] = t

            for t4 in range(kpc):  # V: [tok, dk] per 128-token tile
                pp = ps_tile("bk7")
                for f in range(nf):
                    nc.tensor.matmul(pp[:, t4 * P:(t4 + 1) * P],
                                     xsl("v", f, co + t4 * P, co + (t4 + 1) * P),
                                     wv_sb[:, f, :],
                                     start=(f == 0), stop=(f == nf - 1))
                kt = jj * kpc + t4
                for h in range(HPC):
                    vt = persist.tile([P, 65], F16, tag=f"v{h}_{bb}_{kt}",
                                      name=f"v{h}_{bb}_{kt}")
                    nc.vector.memset(vt[:, 64:65], 1.0)
                    nc.vector.tensor_copy(
                        vt[:, 0:64], pp[:, t4 * P + h * 64:t4 * P + h * 64 + 64])
                    v_tiles[(bb, kt, h)] = vt

            # norm + Wo of the PREVIOUS chunk goes here: its reciprocal/
            # broadcast chain hides behind this chunk's projection matmuls.
            if pending_norm_wo is not None:
                pending_norm_wo()
                pending_norm_wo = None

            # ---- attention for chunk (bb, jj) ----------------------------
            # Software-pipelined emission: QK/exp of k-tile kt+1 is emitted
            # BEFORE PV of k-tile kt, so the in-order PE always has matmul
            # work while the ACT engine runs exp.
            i = jj
            ctx_t = persist.tile([P, QC], F16, tag=f"ctx{j % 2}",
                                 name=f"ctx{j % 2}")
            pc = {0: ps_tile("bk4"), 1: ps_tile("bk5")}
            nkt_i = kpc * (i + 1)
            qtile = qt_tiles[j]

            def emit_qk_exp(kt, i=i, bb=bb, qtile=qtile):
                """QK matmuls + exp for k-tile kt; returns PV emit closure."""
                jk = bb * nch + kt // kpc
                ko = (kt % kpc) * P
                tdiag = kt - kpc * i
                ktile = kt_tiles[jk]
                sw = ps_tile("swA" if kt % 2 == 0 else "swB", width=2 * QC)
                if tdiag < 0:
                    for h in range(HPC):
                        nc.tensor.matmul(sw[:, h * QC:(h + 1) * QC],
                                         ktile[h * 64:h * 64 + 64, ko:ko + P],
                                         qtile[h * 64:h * 64 + 64, :],
                                         start=True, stop=True)
                    ptw = pts.tile([P, 2 * QC], F16, tag="ptw", name="ptw")
                    nc.scalar.activation(ptw[:], sw[:], Exp, scale=0.125)
                    pv_in = {h: ptw[:, h * QC:(h + 1) * QC] for h in range(HPC)}
                    c0 = 0
                else:
                    # diagonal k-tile: h0 scores land at [c0:QC], h1 at
                    # [QC:2*QC-c0] (shifted left so one exp covers both)
                    c0 = P * tdiag
                    ptag = f"ptd{tdiag}"
                    pt = ptd.tile([P, 2 * QC], F16, tag=ptag, name=ptag)
                    nc.tensor.matmul(sw[:, c0:QC],
                                     ktile[0:64, ko:ko + P],
                                     qtile[0:64, c0:QC], start=True, stop=True)
                    nc.tensor.matmul(sw[:, QC:2 * QC - c0],
                                     ktile[64:128, ko:ko + P],
                                     qtile[64:128, c0:QC], start=True, stop=True)
                    if c0 > 0 and ptag not in diag_zeroed:
                        nc.vector.memset(pt[:, 0:c0], 0.0)
                        diag_zeroed.add(ptag)
                    nc.scalar.activation(pt[:, c0:2 * QC - c0],
                                         sw[:, c0:2 * QC - c0], Exp, scale=0.125)
                    nc.vector.tensor_mul(pt[:, c0:c0 + P], pt[:, c0:c0 + P],
                                         tri_sb[:])
                    nc.vector.tensor_mul(pt[:, QC:QC + P], pt[:, QC:QC + P],
                                         tri_sb[:])
                    pv_in = {0: pt[:, c0:QC], 1: pt[:, QC:2 * QC - c0]}

                def emit_pv(kt=kt, pv_in=pv_in, c0=c0, bb=bb, pc=pc,
                            nkt_i=nkt_i):
                    for h in range(HPC):
                        vt = v_tiles[(bb, kt, h)]
                        nc.tensor.matmul(pc[h][0:65, c0:QC], vt[:], pv_in[h],
                                         start=(kt == 0),
                                         stop=(kt == nkt_i - 1))
                return emit_pv

            pv_prev = emit_qk_exp(0)
            for kt in range(1, nkt_i):
                pv_next = emit_qk_exp(kt)
                pv_prev()
                pv_prev = pv_next
            pv_prev()

            def norm_wo(bb=bb, i=i, j=j, pc=pc, ctx_t=ctx_t):
                # normalize: ctx^T[h] *= 1/sums[h] (partition-broadcast via a
                # K=1 PE matmul with a ones row)
                # normalize: ctx[h] rows 0:64 scaled by 1/sums (row 64).
                # Broadcast 1/sums across partitions with a K=1 fp16 matmul;
                # ACT moves it to SBUF so the multiply reads only one PSUM
                # operand. h1's result is lifted to partitions 64:128 of
                # ctx_t with a small SBUF->SBUF DMA (cross-partition moves
                # are DMA-only).
                for h in range(HPC):
                    rc32 = rcs.tile([P, QC], F32, tag="rc32", name="rc32")
                    nc.vector.reciprocal(rc32[64:65, :], pc[h][64:65, :])
                    rc16 = rcs.tile([P, QC], F16, tag="rc16", name="rc16")
                    nc.vector.tensor_copy(rc16[64:65, :], rc32[64:65, :])
                    bc = ps_tile("bk6")
                    bc_sb = rcs.tile([P, QC], F16, tag="bcsb", name="bcsb")
                    nc.tensor.matmul(bc[0:64, :], ones_sb[64:65, :],
                                     rc16[64:65, :], start=True, stop=True)
                    nc.scalar.copy(bc_sb[0:64, :], bc[0:64, :])
                    if h == 0:
                        nc.vector.tensor_mul(ctx_t[0:64, :], pc[0][0:64, :],
                                             bc_sb[0:64, :])
                    else:
                        ctxh1 = rcs.tile([P, QC], F16, tag="ctxh1",
                                         name="ctxh1")
                        nc.vector.tensor_mul(ctxh1[0:64, :], pc[1][0:64, :],
                                             bc_sb[0:64, :])
                        nc.sync.dma_start(ctx_t[64:128, :], ctxh1[0:64, :])

                # output projection for this chunk (bank bk6; bk7 stays free
                # for the next chunk's projections)
                for m in range(nf):
                    po = ps_tile("bk6")
                    nc.tensor.matmul(po[:], wo_sb[:, m * P:(m + 1) * P], ctx_t[:],
                                     start=True, stop=True)
                    ot = wout.tile([P, QC], F16, tag="wo_out", name="wo_out")
                    nc.vector.tensor_copy(ot[:], po[:])
                    nc.gpsimd.dma_start(
                        out_t[m * P:(m + 1) * P,
                              bb * s + i * QC: bb * s + (i + 1) * QC],
                        ot[:])

            pending_norm_wo = norm_wo

    pending_norm_wo()


def build_program(s=S, d=D, b=B):
    import concourse.tile as tile
    from concourse import bacc, mybir
    from contextlib import ExitStack

    F16 = mybir.dt.float16
    F32 = mybir.dt.float32
    bs = b * s

    nc = bacc.Bacc("TRN2", target_bir_lowering=False, debug=False)
    io = {
        "xq_t": nc.dram_tensor("xq_t", [d, bs], F16, kind="ExternalInput").ap(),
        "xk_t": nc.dram_tensor("xk_t", [d, bs], F16, kind="ExternalInput").ap(),
        "xv_t": nc.dram_tensor("xv_t", [d, bs], F16, kind="ExternalInput").ap(),
        "wq": nc.dram_tensor("wq", [d, P], F16, kind="ExternalInput").ap(),
        "wk": nc.dram_tensor("wk", [d, P], F16, kind="ExternalInput").ap(),
        "wv": nc.dram_tensor("wv", [d, P], F16, kind="ExternalInput").ap(),
        "wo": nc.dram_tensor("wo", [P, d], F16, kind="ExternalInput").ap(),
        "bq": nc.dram_tensor("bq", [P, 1], F32, kind="ExternalInput").ap(),
        "bk": nc.dram_tensor("bk", [P, 1], F32, kind="ExternalInput").ap(),
        "tri": nc.dram_tensor("tri", [P, P], F16, kind="ExternalInput").ap(),
        "out_t": nc.dram_tensor("out_t", [d, bs], F16, kind="ExternalOutput").ap(),
    }
    with tile.TileContext(nc) as tc, ExitStack() as ctx:
        _mha_body(ctx, tc, io, s, d, b)
    nc.compile()
    return nc


# ---------------------------------------------------------------------------
# Host side
# ---------------------------------------------------------------------------

def _np_reference(query, key, value, mask, Wq, bq, Wk, bk, Wv, bv, Wo, bo):
    """Pure-numpy fallback, exact reference math (used only if the mask is
    not the expected causal mask)."""
    q = (query.reshape(-1, D) @ Wq + bq).reshape(B, S, H, DK).transpose(0, 2, 1, 3)
    k = (key.reshape(-1, D) @ Wk + bk).reshape(B, S, H, DK).transpose(0, 2, 1, 3)
    v = (value.reshape(-1, D) @ Wv + bv).reshape(B, S, H, DK).transpose(0, 2, 1, 3)
    scores = np.einsum("bhqd,bhkd->bhqk", q, k) / math.sqrt(DK)
    scores = np.where(mask[:, None, :, :] == 0, np.float32(-1e9), scores)
    scores -= scores.max(axis=-1, keepdims=True)
    p = np.exp(scores)
    p /= p.sum(axis=-1, keepdims=True)
    x = np.einsum("bhqk,bhkd->bhqd", p, v)
    x = x.transpose(0, 2, 1, 3).reshape(B, -1, D)
    return (x @ Wo + bo).astype(np.float32)


def _shard_inputs(query, key, value, Wq, bq, Wk, bk, Wv, Wo):
    f16 = np.float16
    xq_t = np.ascontiguousarray(query.reshape(B * S, D).T).astype(f16)
    xk_t = np.ascontiguousarray(key.reshape(B * S, D).T).astype(f16)
    xv_t = np.ascontiguousarray(value.reshape(B * S, D).T).astype(f16)
    idx = np.arange(P)
    tri = (idx[:, None] <= idx[None, :]).astype(f16)  # tri[k, q] = k <= q
    in_maps = []
    for c in range(NCORES):
        sl = slice(c * HPC * DK, (c + 1) * HPC * DK)
        in_maps.append({
            "xq_t": xq_t,
            "xk_t": xk_t,
            "xv_t": xv_t,
            "wq": np.ascontiguousarray(Wq[:, sl]).astype(f16),
            "wk": np.ascontiguousarray(Wk[:, sl]).astype(f16),
            "wv": np.ascontiguousarray(Wv[:, sl]).astype(f16),
            "wo": np.ascontiguousarray(Wo[sl, :]).astype(f16),
            "bq": np.ascontiguousarray(bq[sl]).reshape(P, 1).astype(np.float32),
            "bk": np.ascontiguousarray(bk[sl]).reshape(P, 1).astype(np.float32),
            "tri": tri,
        })
    return in_maps


def kernel(**inputs):
    query = np.asarray(inputs["query"], np.float32)
    key = np.asarray(inputs["key"], np.float32)
    value = np.asarray(inputs["value"], np.float32)
    mask = np.asarray(inputs["mask"])
    Wq = np.asarray(inputs["Wq"], np.float32)
    bq = np.asarray(inputs["bq"], np.float32)
    Wk = np.asarray(inputs["Wk"], np.float32)
    bk = np.asarray(inputs["bk"], np.float32)
    Wv = np.asarray(inputs["Wv"], np.float32)
    bv = np.asarray(inputs["bv"], np.float32)
    Wo = np.asarray(inputs["Wo"], np.float32)
    bo = np.asarray(inputs["bo"], np.float32)

    # The device program hardcodes causal structure; verify and fall back
    # to exact host math for any other mask.
    tril = np.tril(np.ones((S, S), np.int8))
    if mask.shape != (B, S, S) or not np.array_equal(
            (mask != 0).astype(np.int8), np.broadcast_to(tril, (B, S, S))):
        return _np_reference(query, key, value, mask,
                             Wq, bq, Wk, bk, Wv, bv, Wo, bo)

    in_maps = _shard_inputs(query, key, value, Wq, bq, Wk, bk, Wv, Wo)
    outs = _run_spmd(in_maps)

    acc = outs.astype(np.float32).sum(axis=0)  # [D, B*S]
    out = acc.T + (bv @ Wo + bo)[None, :]
    return out.reshape(B, S, D).astype(np.float32)


def _get_exec():
    """Build (once) the program + jitted SPMD executable."""
    if "exec" in _PROGRAM_CACHE:
        return _PROGRAM_CACHE["exec"]
    import jax
    from jax.sharding import Mesh, PartitionSpec
    from jax.experimental.shard_map import shard_map
    import concourse.mybir as mybir
    from concourse import bass2jax

    nc = build_program()
    _PROGRAM_CACHE["nc"] = nc
    bass2jax.install_neuronx_cc_hook()
    partition_name = nc.partition_id_tensor.name if nc.partition_id_tensor else None
    in_names, out_names, out_avals, zero_outs = [], [], [], []
    for alloc in nc.m.functions[0].allocations:
        if not isinstance(alloc, mybir.MemoryLocationSet):
            continue
        name = alloc.memorylocations[0].name
        if alloc.kind == "ExternalInput":
            if name != partition_name:
                in_names.append(name)
        elif alloc.kind == "ExternalOutput":
            out_names.append(name)
            shape = tuple(alloc.tensor_shape)
            dtype = mybir.dt.np(alloc.dtype)
            out_avals.append(jax.core.ShapedArray(shape, dtype))
            zero_outs.append(np.zeros(shape, dtype))
    n_params = len(in_names)
    all_in_names = list(in_names) + list(out_names)
    if partition_name is not None:
        all_in_names.append(partition_name)

    def _body(*args):
        operands = list(args)
        if partition_name is not None:
            operands.append(bass2jax.partition_id_tensor())
        return tuple(bass2jax._bass_exec_p.bind(
            *operands,
            out_avals=tuple(out_avals),
            in_names=tuple(all_in_names),
            out_names=tuple(out_names),
            lowering_input_output_aliases=(),
            sim_require_finite=True,
            sim_require_nnan=True,
            nc=nc,
        ))

    devices = jax.devices()[:NCORES]
    assert len(devices) >= NCORES, f"need {NCORES} neuron cores, have {len(devices)}"
    mesh = Mesh(np.asarray(devices[:NCORES]), ("core",))
    fn = jax.jit(
        shard_map(_body, mesh=mesh,
                  in_specs=(PartitionSpec("core"),) * (n_params + len(zero_outs)),
                  out_specs=(PartitionSpec("core"),) * len(out_names),
                  check_rep=False),
        donate_argnums=tuple(range(n_params, n_params + len(out_names))),
        keep_unused=True)
    _PROGRAM_CACHE["exec"] = (fn, in_names, zero_outs)
    return _PROGRAM_CACHE["exec"]


def _run_spmd(in_maps):
    """Run the SPMD program on 8 cores; returns per-core out_t [8, D, B*S]."""
    fn, in_names, zero_outs = _get_exec()
    concat_in = [np.concatenate([np.asarray(in_maps[c][nm])
                                 for c in range(NCORES)], axis=0)
                 for nm in in_names]
    concat_zero = [np.zeros((NCORES * z.shape[0], *z.shape[1:]), z.dtype)
                   for z in zero_outs]
    out = fn(*concat_in, *concat_zero)
    LAST["out"] = out
    return np.asarray(out[0]).reshape(NCORES, D, B * S)
